# revision 1
# baseline (speedup 1.0000x reference)
"""KeyedGRU (nn_KeyedEncoder) Trainium2 Bass kernel.

Sharding: data-parallel over batch (16 rows/core on 8 NeuronCores); GRU
weights + embedding table replicated. Per core, the 512-step recurrence runs
with gates in PSUM [16, 3072] (batch on partitions), the stationary matmul
operand hT rebuilt each step via PE transposes, and the input projection
precomputed in fused 16-step chunks straight from an indirect-DMA embedding
gather (token layout puts 128 tokens = 8 steps x 16 batch on partitions, so
the gather lands transpose-ready). Matmuls run in bf16 with fp32 PSUM
accumulation; all elementwise math is fp32.
"""
import sys

for p in ("/opt/trn_rl_repo", "/root/.axon_site/_ro/trn_rl_repo"):
    if p not in sys.path:
        sys.path.append(p)

from contextlib import ExitStack

import numpy as np
import ml_dtypes

import concourse.bass as bass
import concourse.tile as tile
from concourse import bacc, mybir
from concourse.bass import IndirectOffsetOnAxis

BF16 = mybir.dt.bfloat16
F32 = mybir.dt.float32
I32 = mybir.dt.int32
AF = mybir.ActivationFunctionType
OP = mybir.AluOpType
ds, ts = bass.ds, bass.ts

V, E, H, B, T = 50000, 300, 1024, 128, 512
KEY_LEN = 15
H3 = 3 * H
EP = 384           # padded E: 300 data + 1 bias-ones column + 83 zeros
NCORES = 8
BL = B // NCORES   # 16 batch rows per core
SPM = 128 // BL    # steps per m-tile = 8
MT = 2             # m-tiles per chunk
C = SPM * MT       # steps per xproj chunk = 16


def _host_prep(inputs):
    bf = ml_dtypes.bfloat16
    emb = np.asarray(inputs["emb"], np.float32)
    W_ih = np.asarray(inputs["W_ih"], np.float32)
    W_hh = np.asarray(inputs["W_hh"], np.float32)
    b_ih = np.asarray(inputs["b_ih"], np.float32)
    b_hh = np.asarray(inputs["b_hh"], np.float32)
    W_g = np.asarray(inputs["W_g"], np.float32)
    b_g = np.asarray(inputs["b_g"], np.float32)
    x = np.asarray(inputs["x"]).astype(np.int32)
    key_ids = np.asarray(inputs["key_ids"]).astype(np.int32).reshape(-1)

    whhT = np.ascontiguousarray(
        W_hh.T.reshape(8, 128, H3).transpose(1, 0, 2)).reshape(128, 8 * H3)
    wpad = np.zeros((EP, H3), np.float32)
    wpad[:E] = W_ih.T
    wpad[E] = b_ih + b_hh          # both biases folded into the ones column
    wihT = np.ascontiguousarray(
        wpad.reshape(3, 128, H3).transpose(1, 0, 2)).reshape(128, 3 * H3)
    wgT = np.ascontiguousarray(
        W_g.T.reshape(8, 128, H).transpose(1, 0, 2)).reshape(128, 8 * H)

    shared = dict(
        emb=emb.astype(bf),
        whhT=whhT.astype(bf),
        wihT=wihT.astype(bf),
        wgT=wgT.astype(bf),
        bg=b_g.reshape(1, H).copy(),
        kidx=key_ids.reshape(KEY_LEN, 1).copy(),
        ident=np.eye(128, dtype=bf),
    )
    per_core = []
    for c in range(NCORES):
        xs = x[c * BL:(c + 1) * BL]
        # xidx[cc*BL + b, j] = xs[b, j*SPM + cc]  (m-tile j, step-in-tile cc)
        xi = xs.reshape(BL, T // SPM, SPM).transpose(2, 0, 1)
        d = dict(shared)
        d["xidx"] = np.ascontiguousarray(xi.reshape(SPM * BL, T // SPM))
        per_core.append(d)
    return per_core


def _build(nsteps=T, key_steps=KEY_LEN):
    nc = bacc.Bacc("TRN2", target_bir_lowering=False, debug=False,
                   num_devices=NCORES)

    dram = {}
    for name, shape, dt in [
        ("emb", [V, E], BF16), ("whhT", [128, 8 * H3], BF16),
        ("wihT", [128, 3 * H3], BF16), ("wgT", [128, 8 * H], BF16),
        ("bg", [1, H], F32), ("kidx", [KEY_LEN, 1], I32),
        ("ident", [128, 128], BF16), ("xidx", [SPM * BL, T // SPM], I32),
    ]:
        dram[name] = nc.dram_tensor(name, shape, dt, kind="ExternalInput")
    out_d = nc.dram_tensor("out", [BL, H], F32, kind="ExternalOutput")

    nchunks = max(1, nsteps // C)

    with tile.TileContext(nc) as tc, ExitStack() as ctx:
        consts = ctx.enter_context(tc.tile_pool(name="consts", bufs=1))
        sb = ctx.enter_context(tc.tile_pool(name="sb", bufs=2))
        ew = ctx.enter_context(tc.tile_pool(name="ew", bufs=2))
        pers = ctx.enter_context(tc.tile_pool(name="pers", bufs=1))
        psg = ctx.enter_context(tc.tile_pool(name="psg", bufs=1, space="PSUM"))
        pss = ctx.enter_context(tc.tile_pool(name="pss", bufs=2, space="PSUM"))

        keyctx = ExitStack()
        keypool = keyctx.enter_context(tc.tile_pool(name="key", bufs=1))
        dpool = keyctx.enter_context(
            tc.tile_pool(name="kdram", bufs=1, space="DRAM"))

        cst = {}
        for name in ["whhT", "wihT", "ident", "kidx", "xidx"]:
            t = consts.tile(dram[name].shape, dram[name].dtype, tag=name)
            nc.sync.dma_start(t, dram[name].ap())
            cst[name] = t
        for name in ["wgT", "bg"]:
            t = keypool.tile(dram[name].shape, dram[name].dtype, tag=name)
            nc.sync.dma_start(t, dram[name].ap())
            cst[name] = t
        whh, wih, wg = cst["whhT"], cst["wihT"], cst["wgT"]
        ident = cst["ident"]

        hT = pers.tile([128, 8 * BL], BF16, tag="hT")
        h = pers.tile([BL, H], F32, tag="h")
        khT = pers.tile([128, 8], BF16, tag="khT")
        kh = pers.tile([1, H], F32, tag="kh")
        gb = pers.tile([BL, H], F32, tag="gb")
        kxp = dpool.tile([KEY_LEN, H3], F32, tag="kxp")
        nc.vector.memset(hT, 0.0)
        nc.vector.memset(h, 0.0)
        nc.vector.memset(khT, 0.0)
        nc.vector.memset(kh, 0.0)

        emb_ap = dram["emb"].ap()

        # =========== key encoder ===========
        kxe = keypool.tile([KEY_LEN, EP], BF16, tag="kxe")
        nc.vector.memset(kxe[:, E:], 0.0)
        nc.gpsimd.indirect_dma_start(
            out=kxe[:, 0:E], out_offset=None,
            in_=emb_ap, in_offset=IndirectOffsetOnAxis(ap=cst["kidx"][:], axis=0))
        nc.vector.memset(kxe[:, E:E + 1], 1.0)

        kxeT = keypool.tile([128, 3 * KEY_LEN], BF16, tag="kxeT")
        for k in range(3):
            ptr = pss.tile([128, 128], BF16, tag="scratch")
            nc.tensor.transpose(ptr[:, 0:KEY_LEN], kxe[:, ts(k, 128)],
                                ident[0:KEY_LEN, 0:KEY_LEN])
            nc.vector.tensor_copy(kxeT[:, ts(k, KEY_LEN)], ptr[:, 0:KEY_LEN])

        for n in range(6):
            pxp = pss.tile([128, 512], F32, tag="scratch")
            for k in range(3):
                nc.tensor.matmul(pxp[0:KEY_LEN, :], kxeT[:, ts(k, KEY_LEN)],
                                 wih[:, k * H3 + n * 512: k * H3 + (n + 1) * 512],
                                 start=(k == 0), stop=(k == 2))
            kxs = keypool.tile([KEY_LEN, 512], F32, tag="kxs", bufs=2)
            nc.vector.tensor_copy(kxs, pxp[0:KEY_LEN, :])
            nc.sync.dma_start(kxp[:, ts(n, 512)], kxs)

        for t in range(key_steps):
            kxt = keypool.tile([1, H3], F32, tag="kxt", bufs=2)
            nc.sync.dma_start(kxt, kxp[t:t + 1, :])
            _gru_step(nc, psg, pss, ew, P=1, hT=khT, h=kh,
                      whh=whh, xp=kxt, gate=None, ident=ident)

        # key gate: gb = sigmoid(kh @ W_g.T + b_g), broadcast over batch rows
        pgt = psg.tile([BL, H3], F32, tag="gates")
        for n in range(2):
            for k in range(8):
                nc.tensor.matmul(pgt[0:1, ts(n, 512)], khT[:, k:k + 1],
                                 wg[:, k * H + n * 512: k * H + (n + 1) * 512],
                                 start=(k == 0), stop=(k == 7))
        grow = keypool.tile([1, H], F32, tag="grow")
        nc.vector.tensor_add(grow, pgt[0:1, 0:H], cst["bg"][:])
        nc.scalar.activation(grow, grow, AF.Sigmoid)
        for p in range(BL):
            nc.sync.dma_start(gb[p:p + 1, :], grow)
        keyctx.close()

        # =========== main sequence ===========
        def emit_chunk_prep(ci):
            xe = sb.tile([128, MT, EP], BF16, tag="xe")
            nc.vector.memset(xe[:, :, E:], 0.0)
            for m in range(MT):
                nc.gpsimd.indirect_dma_start(
                    out=xe[:, m, 0:E], out_offset=None,
                    in_=emb_ap,
                    in_offset=IndirectOffsetOnAxis(
                        ap=cst["xidx"][:, ci * MT + m: ci * MT + m + 1],
                        axis=0))
            nc.vector.memset(xe[:, :, E:E + 1], 1.0)

            xpj = sb.tile([128, MT * H3], BF16, tag="xpj")
            for m in range(MT):
                xeT = sb.tile([128, 3 * 128], BF16, tag="xeT")
                for k in range(3):
                    ptr = pss.tile([128, 128], BF16, tag="scratch")
                    nc.tensor.transpose(ptr, xe[:, m, ts(k, 128)], ident)
                    nc.vector.tensor_copy(xeT[:, ts(k, 128)], ptr)
                for n in range(6):
                    pxp = pss.tile([128, 512], F32, tag="scratch")
                    for k in range(3):
                        nc.tensor.matmul(
                            pxp, xeT[:, ts(k, 128)],
                            wih[:, k * H3 + n * 512: k * H3 + (n + 1) * 512],
                            start=(k == 0), stop=(k == 2))
                    nc.vector.tensor_copy(xpj[:, ds(m * H3 + n * 512, 512)],
                                          pxp)
            return xpj

        xpj_cur = emit_chunk_prep(0)
        for ci in range(nchunks):
            for cs in range(min(C, nsteps)):
                if cs == SPM // 2 and ci + 1 < nchunks:
                    xpj_next = emit_chunk_prep(ci + 1)
                m, r = cs // SPM, cs % SPM
                xps = sb.tile([BL, H3], BF16, tag="xps", bufs=2)
                nc.sync.dma_start(xps, xpj_cur[ds(r * BL, BL), ds(m * H3, H3)])
                _gru_step(nc, psg, pss, ew, P=BL, hT=hT, h=h, whh=whh,
                          xp=xps, gate=gb, ident=ident)
            if ci + 1 < nchunks:
                xpj_cur = xpj_next

        nc.sync.dma_start(out_d.ap(), h)

    nc.compile()
    return nc


def _gru_step(nc, psg, pss, ew, P, hT, h, whh, xp, gate, ident):
    """One GRU step; P = batch rows on partitions (= hT tile width).
    hT [128, 8*P] bf16; h [P, H] f32 (updated in place); xp [P, H3]."""
    pg = psg.tile([BL, H3], F32, tag="gates")
    for n in range(6):
        for k in range(8):
            nc.tensor.matmul(
                pg[0:P, ts(n, 512)], hT[:, ts(k, P)],
                whh[:, k * H3 + n * 512: k * H3 + (n + 1) * 512],
                start=(k == 0), stop=(k == 7))

    r = ew.tile([P, H], F32, tag="r", bufs=1)
    z = ew.tile([P, H], F32, tag="z", bufs=1)
    nc.vector.tensor_add(pg[0:P, 0:H], pg[0:P, 0:H], xp[:, 0:H])
    nc.scalar.activation(r, pg[0:P, 0:H], AF.Sigmoid)
    nc.vector.tensor_add(pg[0:P, H:2 * H], pg[0:P, H:2 * H], xp[:, H:2 * H])
    nc.scalar.activation(z, pg[0:P, H:2 * H], AF.Sigmoid)

    # h' = gate * ((1-z)*n + z*h) = v*n + u, u = gate*z*h, v = gate*(1-z)
    u = ew.tile([P, H], F32, tag="u", bufs=1)
    v = ew.tile([P, H], F32, tag="v", bufs=1)
    if gate is not None:
        zg = ew.tile([P, H], F32, tag="zg", bufs=1)
        nc.vector.tensor_mul(zg, z, gate[0:P, :])
        nc.vector.tensor_mul(u, zg, h)
        nc.vector.tensor_sub(v, gate[0:P, :], zg)
    else:
        nc.vector.tensor_mul(u, z, h)
        nc.vector.tensor_scalar(v, z, -1.0, 1.0, OP.mult, OP.add)

    nc.vector.tensor_mul(pg[0:P, 2 * H:], pg[0:P, 2 * H:], r)
    nc.vector.tensor_add(pg[0:P, 2 * H:], pg[0:P, 2 * H:], xp[:, 2 * H:])
    nt = ew.tile([P, H], F32, tag="nt")
    nc.scalar.activation(nt, pg[0:P, 2 * H:], AF.Tanh)

    nc.vector.tensor_mul(nt, nt, v)
    nc.vector.tensor_add(h, nt, u)

    hbf = ew.tile([P, H], BF16, tag="hbf", bufs=1)
    nc.vector.tensor_copy(hbf, h)
    for k in range(8):
        ptr = pss.tile([128, 128], BF16, tag="scratch")
        nc.tensor.transpose(ptr[:, 0:P], hbf[:, ts(k, 128)], ident[0:P, 0:P])
        nc.vector.tensor_copy(hT[:, ts(k, P)], ptr[:, 0:P])


_NC_CACHE = {}


def kernel(**inputs) -> np.ndarray:
    from concourse.bass_utils import run_bass_kernel_spmd

    per_core = _host_prep(inputs)
    if "nc" not in _NC_CACHE:
        _NC_CACHE["nc"] = _build()
    nc = _NC_CACHE["nc"]
    res = run_bass_kernel_spmd(nc, per_core, core_ids=list(range(NCORES)))
    out = np.concatenate([res.results[c]["out"] for c in range(NCORES)],
                         axis=0)
    return out.astype(np.float32)
